# revision 1
# baseline (speedup 1.0000x reference)
"""MoE routing kernel for TRN2 (8 NeuronCores), Bass/Tile.

Data-parallel over batch (8 samples/core, all 4 gates). Host computes the
gating with eager jnp ops mirroring the reference op-for-op (bit-exact
routing). Key optimization: after folding the BN scale and gate weight tw
into the W3 panel, the mm1+mm2 stages depend only on (sample, expert), so
chains from different gates that route the same sample to the same expert
share one "slot" computation (~43 distinct of 64 per core). Slot h2
results live in an SBUF ring; the per-(gate,sample) combine stage (mm3)
reads its two slots through register-offset (dynamic) APs, with slot ids
loaded from an int32 side input. The Tile program is fully static and
reused across calls (compiled once, lru-cached). All matmuls in float32r.

Per slot (s,e):  h1 = W1[e]@X[s] + b1[e]         (PSUM -> SBUF f32r)
                 h2 = relu(W2[e]@h1 + biasA/inv) (-> h2 ring, f32r)
Per (g,s) pair:  psY = sum_t (W3[e_t].T scaled by inv*tw_t) @ h2[slot_t]
                 out = psY + sum_t tw_t*b3[e_t]
"""
import functools

import numpy as np

E, TOP, C, HD, B, H, W_, NG = 8, 2, 128, 256, 64, 32, 32, 4
P = H * W_            # 1024
NCORES = 8
SPC = B // NCORES     # samples per core: 8
CHAINS = SPC * NG     # (g,s) pairs per core: 32
EPS = 1e-5
NH = 512              # matmul free-dim chunk

NSLOT = 48            # static slot budget per core
# end-exclusive static slot boundary per sample: all slots for sample s
# must occupy indices < BOUND[s] (and >= BOUND[s]-21 for ring safety)
BOUND = [7, 14, 21, 27, 33, 38, 43, 48]
RING = 24             # h2 ring depth (slots, fp16)
WSC = 768             # slot panel cols: W1T(256)|W2T_k0(256)|W2T_k1(256)
MCOLS = 4 * NSLOT + CHAINS          # per-slot biases + per-pair bias3
SIDXC = 2 * NSLOT + 8 * CHAINS      # int32 offsets: xs per slot, h2 per pair


@functools.lru_cache(maxsize=1)
def _build_program():
    from concourse import bacc, mybir
    import concourse.bass as bass
    import concourse.tile as tile

    f32 = mybir.dt.float32
    f32r = mybir.dt.float32r
    i32 = mybir.dt.int32
    nc = bacc.Bacc("TRN2", target_bir_lowering=False, debug=False)

    xq_d = nc.dram_tensor("xq", [SPC, C, P], f32r, kind="ExternalInput")
    ws_d = nc.dram_tensor("ws", [NSLOT, C, WSC], f32r, kind="ExternalInput")
    w3_d = nc.dram_tensor("w3", [2 * CHAINS, C, 256], mybir.dt.float16,
                          kind="ExternalInput")
    meta_d = nc.dram_tensor("meta", [C, MCOLS], f32, kind="ExternalInput")
    sidx_d = nc.dram_tensor("sidx", [1, SIDXC], i32, kind="ExternalInput")
    out_d = nc.dram_tensor("out", [CHAINS, C, P], f32, kind="ExternalOutput")

    with tile.TileContext(nc) as tc:
        with tc.tile_pool(name="big", bufs=1) as bigpool, \
             tc.tile_pool(name="ws", bufs=7) as wspool, \
             tc.tile_pool(name="w3", bufs=8) as w3pool, \
             tc.tile_pool(name="h1", bufs=3) as h1pool, \
             tc.tile_pool(name="osb", bufs=3) as opool, \
             tc.tile_pool(name="xq", bufs=3) as xqpool, \
             tc.tile_pool(name="ps", bufs=4, space="PSUM") as pspool:

            h2_all = bigpool.tile([C, RING * 2 * P], mybir.dt.float16,
                                  name="h2_all")

            regs = [nc.alloc_register(mybir.EngineType.PE, name=f"off{i}")
                    for i in range(16)]

            xq_tiles = {}

            def load_xq(q):
                xt = xqpool.tile([C, P], f32r, tag="xq", name="xqt")
                nc.gpsimd.dma_start(out=xt[:], in_=xq_d[q])
                xq_tiles[q] = xt

            ws_tiles = {}

            def load_ws(d):
                t = wspool.tile([C, WSC], f32r, tag="ws", name="wst")
                nc.gpsimd.dma_start(out=t[:], in_=ws_d[d])
                ws_tiles[d] = t

            sidx = bigpool.tile([1, SIDXC], i32, name="sidx")
            nc.gpsimd.dma_start(out=sidx[:], in_=sidx_d[:])
            meta = bigpool.tile([C, MCOLS], f32, name="meta")
            nc.gpsimd.dma_start(out=meta[:], in_=meta_d[:])
            load_xq(0)
            load_ws(0)
            load_xq(1)
            load_ws(1)
            load_ws(2)

            def slot_mm1(d, q):
                """mm1 of slot d (static xs of its schedule region q)."""
                wst = ws_tiles.pop(d)
                xt = xq_tiles[q]
                ps1 = [pspool.tile([C, P], f32, tag="ps", name=f"ps1_{m}")
                       for m in range(2)]
                for m in range(2):
                    lhs = wst[:, m * 128:(m + 1) * 128]
                    for n in range(2):
                        nc.tensor.matmul(
                            ps1[m][:, n * NH:(n + 1) * NH], lhs,
                            xt[:, n * NH:(n + 1) * NH],
                            start=True, stop=True)
                h1t = h1pool.tile([C, 2 * P], f32r, tag="h1", name="h1t")
                b1ap0 = meta[:, 4 * d + 0:4 * d + 1]
                nc.vector.tensor_scalar_add(
                    out=h1t[:, 0:P], in0=ps1[0][:], scalar1=b1ap0)
                b1ap1 = meta[:, 4 * d + 1:4 * d + 2]
                nc.scalar.activation(
                    out=h1t[:, P:2 * P], in_=ps1[1][:],
                    func=mybir.ActivationFunctionType.Identity,
                    bias=b1ap1, scale=1.0)
                return wst, h1t

            def slot_mm2(d, wst, h1t):
                """mm2 of slot d; h2 goes to its static ring position."""
                rp = d % RING
                ps2 = [pspool.tile([C, P], f32, tag="ps", name=f"ps2_{m}")
                       for m in range(2)]
                for m in range(2):
                    for k in range(2):
                        lhs = wst[:, 256 + k * 256 + m * 128:
                                  256 + k * 256 + (m + 1) * 128]
                        for n in range(2):
                            nc.tensor.matmul(
                                ps2[m][:, n * NH:(n + 1) * NH], lhs,
                                h1t[:, k * P + n * NH:k * P + (n + 1) * NH],
                                start=(k == 0), stop=(k == 1))
                base = rp * 2 * P
                bA0 = meta[:, 4 * d + 2:4 * d + 3]
                nc.vector.tensor_scalar(
                    out=h2_all[:, base:base + P], in0=ps2[0][:],
                    scalar1=bA0, scalar2=0.0,
                    op0=mybir.AluOpType.add, op1=mybir.AluOpType.max)
                bA1 = meta[:, 4 * d + 3:4 * d + 4]
                nc.scalar.activation(
                    out=h2_all[:, base + P:base + 2 * P], in_=ps2[1][:],
                    func=mybir.ActivationFunctionType.Relu,
                    bias=bA1, scale=1.0)

            def pair_block(j):
                """mm3 + final for pair j: dynamic h2 ring reads. The 8
                ring offsets for two consecutive pairs load in one 16-reg
                load (pairs pop in consecutive-j order)."""
                w3t = {}
                for t in range(2):
                    w3t[t] = w3pool.tile([C, 256], mybir.dt.float16,
                                         tag="w3", name=f"w3t{t}")
                    nc.gpsimd.dma_start(out=w3t[t][:], in_=w3_d[2 * j + t])
                sb = 2 * NSLOT + 8 * j
                ph = j % 2
                if ph == 0:
                    nend = min(sb + 16, SIDXC)
                    nc.reg_load(regs[0:nend - sb], sidx[0:1, sb:nend])
                psY = pspool.tile([C, P], f32, tag="ps", name="psY")
                for t in range(2):
                    for k in range(2):
                        lhs = w3t[t][:, k * 128:(k + 1) * 128]
                        for n in range(2):
                            off = nc.snap(
                                regs[8 * ph + t * 4 + k * 2 + n],
                                donate=True, min_val=0,
                                max_val=RING * 2 * P - NH)
                            nc.tensor.matmul(
                                psY[:, n * NH:(n + 1) * NH], lhs,
                                h2_all[:, bass.ds(off, NH)],
                                start=(t == 0 and k == 0),
                                stop=(t == 1 and k == 1))
                b3ap = meta[:, 4 * NSLOT + j:4 * NSLOT + j + 1]
                osb = opool.tile([C, P], f32, tag="osb", name="osb")
                nc.vector.tensor_scalar_add(
                    out=osb[:, 0:NH], in0=psY[:, 0:NH], scalar1=b3ap)
                nc.scalar.activation(
                    out=osb[:, NH:P], in_=psY[:, NH:P],
                    func=mybir.ActivationFunctionType.Identity,
                    bias=b3ap, scale=1.0)
                for n in range(2):
                    nc.sync.dma_start(out=out_d[j][:, n * NH:(n + 1) * NH],
                                      in_=osb[:, n * NH:(n + 1) * NH])

            # slots pipelined (mm1 one ahead of mm2); pair blocks of a
            # region interleave 2 iterations after its last slot's mm2 so
            # the conservative whole-tile dep on h2 is already satisfied.
            pair_queue = []   # (ready_iter, j)
            nextq = 0
            pending = None
            for d in range(NSLOT):
                q = next(i for i in range(SPC) if d < BOUND[i])
                if d + 2 < NSLOT:
                    q2 = next(i for i in range(SPC) if d + 2 < BOUND[i])
                    if q2 not in xq_tiles:
                        load_xq(q2)
                if d + 3 < NSLOT and d + 3 not in ws_tiles:
                    load_ws(d + 3)
                wst, h1t = slot_mm1(d, q)
                if pair_queue and d >= pair_queue[0][0]:
                    pair_block(pair_queue.pop(0)[1])
                if pending is not None:
                    slot_mm2(*pending)
                pending = (d, wst, h1t)
                while nextq < SPC and d + 1 >= BOUND[nextq]:
                    pair_queue.extend(
                        (BOUND[nextq] + 2, nextq * NG + g)
                        for g in range(NG))
                    nextq += 1
            slot_mm2(*pending)
            for _, j in pair_queue:
                pair_block(j)

    nc.compile()
    return nc


def _gating(x, gates):
    """Host gating, eager jnp op-for-op as the reference (bit-exact)."""
    import jax
    import jax.numpy as jnp

    xj = jnp.asarray(x)
    gj = jnp.asarray(gates)
    x0 = xj.mean(axis=(2, 3))                      # [B, C]
    tis, tws = [], []
    for i in range(NG):
        probs = jax.nn.softmax(x0 @ gj[i], axis=1)  # [B, E]
        top_p, top_i = jax.lax.top_k(probs, TOP)    # [B, TOP]
        tw = jax.nn.softmax(top_p, axis=1)          # [B, TOP]
        tis.append(np.asarray(top_i))
        tws.append(np.asarray(tw).astype(np.float32))
    return np.stack(tis), np.stack(tws)


def _np_fallback(inputs, top_i, tw):
    """Pure-numpy reference path (only if slot budget overflows)."""
    x = np.asarray(inputs["x"], np.float32).reshape(B, C, P)
    W1 = np.asarray(inputs["W1"], np.float32)
    b1 = np.asarray(inputs["b1"], np.float32)
    W2 = np.asarray(inputs["W2"], np.float32)
    b2 = np.asarray(inputs["b2"], np.float32)
    gmm = np.asarray(inputs["bn_gamma"], np.float32)
    bet = np.asarray(inputs["bn_beta"], np.float32)
    mea = np.asarray(inputs["bn_mean"], np.float32)
    var = np.asarray(inputs["bn_var"], np.float32)
    W3 = np.asarray(inputs["W3"], np.float32)
    b3 = np.asarray(inputs["b3"], np.float32)
    inv = gmm / np.sqrt(var + EPS)
    outs = []
    for g in range(NG):
        og = np.zeros((B, C, P), np.float32)
        for b in range(B):
            for t in range(TOP):
                e = int(top_i[g, b, t])
                w = tw[g, b, t]
                h = W1[e] @ x[b] + b1[e][:, None]
                h = W2[e] @ h + b2[e][:, None]
                h = np.maximum(
                    (h - mea[e][:, None]) * inv[e][:, None]
                    + bet[e][:, None], 0.0)
                og[b] += w * (W3[e] @ h + b3[e][:, None])
        outs.append(og.reshape(B, C, H, W_))
    return tuple(outs)


def build_in_maps(inputs):
    """Gating, slot dedup, packed panels, per-core input maps.

    Returns (in_maps, None), or (None, outputs) for the fallback path."""
    x = np.asarray(inputs["x"], dtype=np.float32)
    gates = np.asarray(inputs["gates"], dtype=np.float32)
    W1 = np.asarray(inputs["W1"], dtype=np.float32)
    b1 = np.asarray(inputs["b1"], dtype=np.float32)
    W2 = np.asarray(inputs["W2"], dtype=np.float32)
    b2 = np.asarray(inputs["b2"], dtype=np.float32)
    bn_gamma = np.asarray(inputs["bn_gamma"], dtype=np.float32)
    bn_beta = np.asarray(inputs["bn_beta"], dtype=np.float32)
    bn_mean = np.asarray(inputs["bn_mean"], dtype=np.float32)
    bn_var = np.asarray(inputs["bn_var"], dtype=np.float32)
    W3 = np.asarray(inputs["W3"], dtype=np.float32)
    b3 = np.asarray(inputs["b3"], dtype=np.float32)

    top_i, tw = _gating(x, gates)  # [NG,B,TOP]

    inv = bn_gamma / np.sqrt(bn_var + np.float32(EPS))   # [E, HD]
    if not np.all(inv > 0):
        return None, _np_fallback(inputs, top_i, tw)
    biasA = (b2 - bn_mean) * inv + bn_beta               # [E, HD]
    bAp = biasA / inv                                    # [E, HD]

    wpanel = np.empty((E, C, WSC), dtype=np.float32)
    w3t_e = np.empty((E, HD, C), dtype=np.float32)
    for e in range(E):
        wpanel[e, :, 0:256] = W1[e].T
        w2t = W2[e].T
        wpanel[e, :, 256:512] = w2t[0:128, :]
        wpanel[e, :, 512:768] = w2t[128:256, :]
        w3t_e[e] = W3[e].T

    xr = x.reshape(B, C, P)
    in_maps = []
    orders = []
    for c in range(NCORES):
        s0 = c * SPC
        # schedule positions process samples densest-first so region
        # demands are non-increasing and fit the static BOUND
        dcount = [len(set(int(top_i[g, s0 + s, t])
                          for g in range(NG) for t in range(TOP)))
                  for s in range(SPC)]
        order = sorted(range(SPC), key=lambda s: -dcount[s])
        orders.append(order)
        slot_of = {}
        slots = [None] * NSLOT
        ok = True
        for q in range(SPC):
            sr = order[q]
            lo = BOUND[q - 1] if q else 0
            for g in range(NG):
                for t in range(TOP):
                    e = int(top_i[g, s0 + sr, t])
                    if (sr, e) in slot_of:
                        continue
                    d = next((i for i in range(lo, BOUND[q])
                              if slots[i] is None), None)
                    if d is None:
                        ok = False
                        break
                    slots[d] = (sr, e)
                    slot_of[(sr, e)] = d
                if not ok:
                    break
            if not ok:
                break
        if not ok:
            return None, _np_fallback(inputs, top_i, tw)
        for q in range(SPC):     # pad free slots with a dup from same region
            lo = BOUND[q - 1] if q else 0
            filler = next((slots[i] for i in range(lo, BOUND[q])
                           if slots[i] is not None), (order[q], 0))
            for i in range(lo, BOUND[q]):
                if slots[i] is None:
                    slots[i] = filler

        ws = np.empty((NSLOT, C, WSC), dtype=np.float32)
        w3 = np.empty((2 * CHAINS, C, 256), dtype=np.float16)
        meta = np.zeros((C, MCOLS), dtype=np.float32)
        sidx = np.zeros((1, SIDXC), dtype=np.int32)
        for d, (s, e) in enumerate(slots):
            ws[d] = wpanel[e]
            sidx[0, 2 * d] = s * P
            sidx[0, 2 * d + 1] = s * P + NH
            meta[:, 4 * d + 0] = b1[e, 0:128]
            meta[:, 4 * d + 1] = b1[e, 128:256]
            meta[:, 4 * d + 2] = bAp[e, 0:128]
            meta[:, 4 * d + 3] = bAp[e, 128:256]
        for q in range(SPC):
            sr = order[q]
            for g in range(NG):
                j = q * NG + g
                bias3 = np.zeros(C, dtype=np.float32)
                sb = 2 * NSLOT + 8 * j
                for t in range(TOP):
                    e = int(top_i[g, s0 + sr, t])
                    w = tw[g, s0 + sr, t]
                    d = slot_of[(sr, e)]
                    rp = d % RING
                    for k in range(2):
                        for n in range(2):
                            sidx[0, sb + t * 4 + k * 2 + n] = (
                                rp * 2 * P + k * P + n * NH)
                    sA = inv[e] * w
                    w3[2 * j + t] = np.hstack(
                        [w3t_e[e][0:128, :] * sA[0:128, None],
                         w3t_e[e][128:256, :] * sA[128:256, None]])
                    bias3 += w * b3[e]
                meta[:, 4 * NSLOT + j] = bias3
        in_maps.append({
            "xq": np.ascontiguousarray(
                xr[[s0 + order[q] for q in range(SPC)]]),
            "ws": ws, "w3": w3, "meta": meta, "sidx": sidx,
        })
    return (in_maps, orders), None


def kernel(x, gates, W1, b1, W2, b2, bn_gamma, bn_beta, bn_mean, bn_var,
           W3, b3):
    from concourse.bass_utils import run_bass_kernel_spmd

    built, fb = build_in_maps({
        "x": x, "gates": gates, "W1": W1, "b1": b1, "W2": W2, "b2": b2,
        "bn_gamma": bn_gamma, "bn_beta": bn_beta, "bn_mean": bn_mean,
        "bn_var": bn_var, "W3": W3, "b3": b3,
    })
    if fb is not None:
        return fb
    in_maps, orders = built
    nc = _build_program()
    res = run_bass_kernel_spmd(nc, in_maps, list(range(NCORES)))

    outs = []
    for g in range(NG):
        og = np.empty((B, C, P), dtype=np.float32)
        for c in range(NCORES):
            for q in range(SPC):
                og[c * SPC + orders[c][q]] = \
                    res.results[c]["out"][q * NG + g]
        outs.append(og.reshape(B, C, H, W_))
    return tuple(outs)

